# revision 18
# baseline (speedup 1.0000x reference)
"""Bass/Trainium2 kernel for nn_BayesianSTDPAdaptive (8-core SPMD).

Math: the reference scan runs 400 steps, but its learning-rate state
collapses after the first step: with mu init == 1.0,
    m1 = 1*x + 0*m1_old = x,  m2 = 1*x*x + 0*m2_old = fl(x*x)
so mu = (m2 - m1*m1)/(exp(-m1)+1) = (fl(x*x) - fl(x*x))/... = exactly 0
for both the weight and bias chains (no-FMA fp32, which is what the
device-executed reference emits: separate mul/add roundings).  All later
steps then add mu*d = 0 — EXCEPT when d overflows to +/-inf, where
0*inf = NaN.  For the weights d is bounded (inputs in [0,1)), so
    w_out = w0 + (corr1 * exp(-w0) - tos1)
For the biases, step t>=1 computes db_t = (exp(-b1)*tos_t - 1)*tos_sum_t
which overflows for very negative b1; those rows become NaN.  The kernel
reproduces the NaN mask exactly by evaluating db_t in fp32 for all
t = 1..399 and testing for overflow.

Sharding: row-parallel over the 256 output neurons, 32 rows per core.
Each core reads the full output_spikes (for the per-step totals), the
first time-batch of input_psp, and its 32-row slice of weights/biases.
The host rotates each core's spike columns so its shard sits at columns
0..31 (the SPMD program is identical on every core).

Raw Bass (no Tile): the walrus build in this container rejects >1 sync
wait on a Drain instruction, which Tile's exit drain always needs, so
semaphores are managed manually: one DMA sem per input tensor (HW-DGE
queues complete out of order) and one counting sem per compute engine.
Each spike chunk is split across the two HW-DGE rings (SP + Activation),
~140 GB/s each.
"""

import sys
from contextlib import ExitStack

for _p in ("/opt/trn_rl_repo", "/root/.axon_site/_ro/trn_rl_repo"):
    if _p not in sys.path:
        sys.path.append(_p)

import numpy as np

import concourse.bass as bass
import concourse.mybir as mybir
from concourse.bass_utils import run_bass_kernel_spmd

# Problem constants (hardcoded per contract)
T, I, O = 4000, 512, 256
TB = 10
ITERS = T // TB          # 400
NCORES = 8
OS = O // NCORES         # 32 output rows per core
FLT_MAX = float(np.finfo(np.float32).max)

F32 = mybir.dt.float32
ALU = mybir.AluOpType
ACTF = mybir.ActivationFunctionType

# t-chunks over the 400 time batches; first is small so compute starts early
CHUNKS = [64, 112, 112, 112]
STARTS = [0, 64, 176, 288]

# --- engine tick numbers (each compute op incs its engine sem by 1) ---
# GpSimd: tmp_k = ticks 1-4.  DVE: memsets 1-3; chunk0 (u,v,tos,tsum) 4-7;
# bias 8-10; mask0 11-14; then per k>=1 chunk 4 ops + mask 3 ops;
# w-chain (p, w1) last so late aux/w0 DMAs never stall the chunk pipeline.
T_TOS0 = 6
T_B1 = 10
T_F = [14, 21, 28, 35]
T_P = 36
T_W1 = 37
T_BO = 40
# ACT
A_EB0, A_TOS1C, A_EBB, A_E = 1, 2, 3, 4
# PE
P_CORR, P_TOS1C, P_EBB, P_CNT = 1, 2, 3, 7


def build_nc(debug=False):
    nc = bass.Bass()

    spk = nc.declare_dram_parameter("spk", [ITERS, TB * O], F32, isOutput=False)
    # aux rows 0-9: [psp1 | spk1s]
    aux = nc.declare_dram_parameter("aux", [TB, I + OS], F32, isOutput=False)
    w0 = nc.declare_dram_parameter("w0", [OS, I], F32, isOutput=False)
    b0 = nc.declare_dram_parameter("b0", [1, OS], F32, isOutput=False)
    w_out = nc.declare_dram_parameter("w_out", [OS, I], F32, isOutput=True)
    b_out = nc.declare_dram_parameter("b_out", [1, OS], F32, isOutput=True)

    dbg = {}
    if debug:
        for name, shape in [
            ("d_st0", [128, TB * O]), ("d_tos0", [128, O]), ("d_tsum0", [128, 1]),
            ("d_e", [OS, I]), ("d_p", [OS, I]), ("d_tos1c", [OS, 1]),
            ("d_eb0", [1, OS]), ("d_y0", [1, OS]), ("d_db", [1, OS]),
            ("d_b1", [1, OS]), ("d_ebb", [128, OS]),
            ("d_yk0", [128, OS]), ("d_vk0", [128, OS]), ("d_fk0", [128, OS]),
            ("d_q", [1, OS]),
        ]:
            dbg[name] = nc.declare_dram_parameter(name, shape, F32, isOutput=True)

    es = ExitStack()
    with es:
        sb = lambda name, shape: es.enter_context(nc.sbuf_tensor(name, shape, F32))
        psumt = lambda name, shape: es.enter_context(nc.psum_tensor(name, shape, F32))

        st = [sb(f"st{k}", [128, TB * O]) for k in range(4)]
        tmp = [sb(f"tmp{k}", [128, 5 * O]) for k in range(4)]
        u = [sb(f"u{k}", [128, 2 * O]) for k in range(4)]
        v = [sb(f"v{k}", [128, O]) for k in range(4)]
        tos = [sb(f"tos{k}", [128, O]) for k in range(4)]
        tsum = [sb(f"tsum{k}", [128, 1]) for k in range(4)]

        aux_t = sb("aux_t", [TB, I + OS])
        psp1_t = aux_t[0:TB, 0:I]
        spk1s_t = aux_t[0:TB, I : I + OS]
        b0_t = sb("b0_t", [1, OS])
        w0_t = sb("w0_t", [OS, I])
        ones_row = sb("ones_row", [1, 128])
        ones_col = sb("ones_col", [128, 1])
        one_t = sb("one_t", [1, 1])

        e_t = sb("e_t", [OS, I])
        p_t = sb("p_t", [OS, I])
        w1_t = sb("w1_t", [OS, I])
        tos1c = sb("tos1c", [OS, 1])
        eb0 = sb("eb0", [1, OS])
        y0 = sb("y0", [1, OS])
        db = sb("db", [1, OS])
        b1_t = sb("b1_t", [1, OS])
        ebb = sb("ebb", [128, OS])
        yk = [sb(f"yk{k}", [128, OS]) for k in range(4)]
        vk = [sb(f"vk{k}", [128, OS]) for k in range(4)]
        fk = [sb(f"fk{k}", [128, OS]) for k in range(4)]
        q_t = sb("q_t", [1, OS])
        nanv = sb("nanv", [1, OS])
        bo_t = sb("bo_t", [1, OS])

        corr_p = psumt("corr_p", [OS, I])
        tos1c_p = psumt("tos1c_p", [OS, 1])
        b1b_p = psumt("b1b_p", [128, OS])
        cnt_p = psumt("cnt_p", [1, OS])

        sem_in = {
            name: es.enter_context(nc.semaphore(f"sem_{name}"))
            for name in ["st0", "st1", "st2", "st3", "aux", "w0", "b0"]
        }
        sem_out = es.enter_context(nc.semaphore("sem_out"))
        dve = es.enter_context(nc.semaphore("sem_dve"))
        act = es.enter_context(nc.semaphore("sem_act"))
        pe = es.enter_context(nc.semaphore("sem_pe"))
        gp = es.enter_context(nc.semaphore("sem_gp"))
        all_sems = list(sem_in.values()) + [sem_out, dve, act, pe, gp]

        ticks = {"dve": 0, "act": 0, "pe": 0, "gp": 0}
        sems = {"dve": dve, "act": act, "pe": pe, "gp": gp}

        def inc(engine_key, inst):
            ticks[engine_key] += 1
            inst.then_inc(sems[engine_key], 1)
            return ticks[engine_key]

        # each spike chunk is split between the SP and ACT HW-DGE rings;
        # both halves inc the same sem by 16, consumers wait >= 32
        def spk_dma_half(eng, k, half):
            tcnt = CHUNKS[k]
            h = tcnt // 2
            lo, hi = (0, h) if half == 0 else (h, tcnt)
            eng.dma_start(
                st[k][lo:hi, :],
                spk[STARTS[k] + lo : STARTS[k] + hi, :],
            ).then_inc(sem_in[f"st{k}"], 16)

        with nc.Block() as block:

            @block.sync
            def _(sync):
                spk_dma_half(sync, 0, 0)
                sync.dma_start(b0_t[:], b0[:]).then_inc(sem_in["b0"], 16)
                sync.dma_start(aux_t[:, :], aux[:]).then_inc(sem_in["aux"], 16)
                for k in range(1, 4):
                    spk_dma_half(sync, k, 0)
                sync.dma_start(w0_t[:], w0[:]).then_inc(sem_in["w0"], 16)
                sync.wait_ge(dve, T_W1)
                sync.dma_start(w_out[:], w1_t[:]).then_inc(sem_out, 16)
                sync.wait_ge(dve, T_BO)
                sync.dma_start(b_out[:], bo_t[:]).then_inc(sem_out, 16)
                n_out = 2
                if dbg:
                    sync.wait_ge(act, A_EBB)
                    sync.wait_ge(pe, P_CNT)
                    for name, tile in [
                        ("d_st0", st[0]), ("d_tos0", tos[0]), ("d_tsum0", tsum[0]),
                        ("d_e", e_t), ("d_p", p_t), ("d_tos1c", tos1c),
                        ("d_eb0", eb0), ("d_y0", y0), ("d_db", db),
                        ("d_b1", b1_t), ("d_ebb", ebb),
                        ("d_yk0", yk[0]), ("d_vk0", vk[0]), ("d_fk0", fk[0]),
                        ("d_q", q_t),
                    ]:
                        sync.dma_start(dbg[name][:], tile[:]).then_inc(sem_out, 16)
                        n_out += 1
                sync.wait_ge(sem_out, 16 * n_out)

            @block.vector
            def _(vector):
                # DVE is pipelined: a same-engine RAW/WAW needs a self-wait
                # on the engine sem (what Tile would auto-insert).
                def vwait():
                    vector.wait_ge(dve, ticks["dve"])

                def chunk_ops(k, tcnt):
                    vector.wait_ge(gp, k + 1)  # tmp_k from GpSimd
                    inc("dve", nc.vector.tensor_tensor(
                        u[k][0:tcnt, :], tmp[k][0:tcnt, 0 : 2 * O],
                        tmp[k][0:tcnt, 2 * O : 4 * O], op=ALU.add))
                    vwait()
                    inc("dve", nc.vector.tensor_tensor(
                        v[k][0:tcnt, :], u[k][0:tcnt, 0:O],
                        u[k][0:tcnt, O : 2 * O], op=ALU.add))
                    vwait()
                    inc("dve", nc.vector.tensor_tensor(
                        tos[k][0:tcnt, :], v[k][0:tcnt, :],
                        tmp[k][0:tcnt, 4 * O : 5 * O], op=ALU.add))
                    vwait()
                    inc("dve", nc.vector.tensor_reduce(
                        tsum[k][0:tcnt, :], tos[k][0:tcnt, :],
                        axis=mybir.AxisListType.X, op=ALU.add))

                def mask_ops(k, tcnt):
                    vwait()
                    inc("dve", nc.vector.tensor_tensor(
                        yk[k][0:tcnt, :], tos[k][0:tcnt, 0:OS], ebb[0:tcnt, :],
                        op=ALU.mult))
                    vwait()
                    inc("dve", nc.vector.tensor_scalar(
                        vk[k][0:tcnt, :], yk[k][0:tcnt, :], 1.0,
                        tsum[k][0:tcnt, 0:1], op0=ALU.subtract, op1=ALU.mult))
                    vwait()
                    t = inc("dve", nc.vector.tensor_single_scalar(
                        fk[k][0:tcnt, :], vk[k][0:tcnt, :], FLT_MAX, op=ALU.is_gt))
                    if k == 0:
                        # t=0 already applied via db; exclude from the mask
                        vwait()
                        t = inc("dve", nc.vector.memset(fk[0][0:1, :], 0.0))
                    assert t == T_F[k]

                t = inc("dve", nc.vector.memset(ones_row[:], 1.0)); assert t == 1
                inc("dve", nc.vector.memset(ones_col[:], 1.0))
                inc("dve", nc.vector.memset(one_t[:], 1.0))
                chunk_ops(0, CHUNKS[0])
                assert ticks["dve"] == T_TOS0 + 1
                # bias chain
                vector.wait_ge(act, A_EB0)
                vwait()
                inc("dve", nc.vector.tensor_tensor(
                    y0[:], eb0[:], tos[0][0:1, 0:OS], op=ALU.mult))
                vwait()
                inc("dve", nc.vector.tensor_scalar(
                    db[:], y0[:], 1.0, tsum[0][0:1, 0:1],
                    op0=ALU.subtract, op1=ALU.mult))
                vwait()
                t = inc("dve", nc.vector.tensor_tensor(
                    b1_t[:], b0_t[:], db[:], op=ALU.add)); assert t == T_B1
                # overflow-mask chunks, interleaved with the remaining DMAs
                vector.wait_ge(act, A_EBB)
                mask_ops(0, CHUNKS[0])
                for k in range(1, 4):
                    chunk_ops(k, CHUNKS[k])
                    mask_ops(k, CHUNKS[k])
                # w-chain last: p = corr * exp(-w0); w1 = (p - tos1) + w0
                vector.wait_ge(act, A_E)
                vector.wait_ge(pe, P_CORR)
                t = inc("dve", nc.vector.tensor_tensor(
                    p_t[:], corr_p[:], e_t[:], op=ALU.mult)); assert t == T_P
                vwait()
                t = inc("dve", nc.vector.scalar_tensor_tensor(
                    w1_t[:], p_t[:], tos1c[:, 0:1], w0_t[:],
                    op0=ALU.subtract, op1=ALU.add)); assert t == T_W1
                # nanify: count>0 -> inf -> inf-inf = NaN
                vector.wait_ge(pe, P_CNT)
                inc("dve", nc.vector.tensor_scalar(
                    q_t[:], cnt_p[:], FLT_MAX, 4.0, op0=ALU.mult, op1=ALU.mult))
                vwait()
                inc("dve", nc.vector.tensor_tensor(
                    nanv[:], q_t[:], q_t[:], op=ALU.subtract))
                vwait()
                t = inc("dve", nc.vector.tensor_tensor(
                    bo_t[:], b1_t[:], nanv[:], op=ALU.add)); assert t == T_BO

            @block.gpsimd
            def _(gpsimd):
                # first-stage pair-sum of each spike chunk on the otherwise
                # idle GpSimd, halving the DVE's per-chunk element work
                for k, tcnt in enumerate(CHUNKS):
                    gpsimd.wait_ge(sem_in[f"st{k}"], 32)
                    if k > 0:
                        gpsimd.wait_ge(gp, k)  # own-pipeline WAW/flush
                    inc("gp", nc.gpsimd.tensor_add(
                        tmp[k][0:tcnt, :], st[k][0:tcnt, 0 : 5 * O],
                        st[k][0:tcnt, 5 * O : 10 * O]))

            @block.scalar
            def _(scalar):
                for k in range(4):
                    spk_dma_half(scalar, k, 1)
                scalar.wait_ge(sem_in["b0"], 16)
                t = inc("act", nc.scalar.activation(
                    eb0[:], b0_t[:], ACTF.Exp, scale=-1.0)); assert t == A_EB0
                scalar.wait_ge(pe, P_TOS1C)
                t = inc("act", nc.scalar.copy(tos1c[:], tos1c_p[:])); assert t == A_TOS1C
                scalar.wait_ge(pe, P_EBB)
                t = inc("act", nc.scalar.activation(
                    ebb[:], b1b_p[:], ACTF.Exp, scale=-1.0)); assert t == A_EBB
                scalar.wait_ge(sem_in["w0"], 16)
                t = inc("act", nc.scalar.activation(
                    e_t[:], w0_t[:], ACTF.Exp, scale=-1.0)); assert t == A_E

            @block.tensor
            def _(tensor):
                tensor.wait_ge(sem_in["aux"], 16)
                t = inc("pe", nc.tensor.matmul(
                    corr_p[:], spk1s_t, psp1_t)); assert t == P_CORR
                tensor.wait_ge(dve, T_TOS0)
                t = inc("pe", nc.tensor.matmul(
                    tos1c_p[:], tos[0][0:1, 0:OS], one_t[:])); assert t == P_TOS1C
                tensor.wait_ge(dve, T_B1)
                t = inc("pe", nc.tensor.matmul(
                    b1b_p[:], ones_row[:], b1_t[:])); assert t == P_EBB
                for k, tcnt in enumerate(CHUNKS):
                    tensor.wait_ge(dve, T_F[k])
                    t = inc("pe", nc.tensor.matmul(
                        cnt_p[:], ones_col[0:tcnt, :], fk[k][0:tcnt, :],
                        start=(k == 0), stop=(k == 3)))
                assert t == P_CNT

        # leave semaphores zeroed so a re-execution of the same loaded NEFF
        # (e.g. a second kernel() call) starts from a clean state
        nc.all_engine_barrier()
        ids = sorted(s.num for s in all_sems)
        assert ids == list(range(ids[0], ids[0] + len(ids))), ids
        nc.gpsimd.sem_clear(range(ids[0], ids[-1] + 1))
        nc.all_engine_barrier()

    return nc


_NC = None


def _get_nc():
    global _NC
    if _NC is None:
        _NC = build_nc()
    return _NC


def make_in_maps(input_psp, output_spikes, weights, biases):
    psp = np.ascontiguousarray(np.asarray(input_psp, dtype=np.float32))
    spk = np.ascontiguousarray(np.asarray(output_spikes, dtype=np.float32))
    w = np.ascontiguousarray(np.asarray(weights, dtype=np.float32))
    b = np.ascontiguousarray(np.asarray(biases, dtype=np.float32))

    in_maps = []
    for c in range(NCORES):
        r0 = c * OS
        # SPMD program is identical on every core, so rotate the spike
        # columns so this core's 32 output rows land at columns 0..31
        perm = np.concatenate(
            [np.arange(r0, r0 + OS), np.delete(np.arange(O), slice(r0, r0 + OS))]
        )
        spk_c = np.ascontiguousarray(spk[:, perm].reshape(ITERS, TB * O))
        aux = np.zeros((TB, I + OS), dtype=np.float32)
        aux[:, :I] = psp[:TB]
        aux[:, I:] = spk[:TB, r0 : r0 + OS]
        in_maps.append(
            {
                "spk": spk_c,
                "aux": aux,
                "w0": np.ascontiguousarray(w[r0 : r0 + OS]),
                "b0": np.ascontiguousarray(b[r0 : r0 + OS].reshape(1, OS)),
            }
        )
    return in_maps


def kernel(input_psp, output_spikes, weights, biases, _trace=False):
    nc = _get_nc()
    in_maps = make_in_maps(input_psp, output_spikes, weights, biases)
    res = run_bass_kernel_spmd(nc, in_maps, core_ids=list(range(NCORES)), trace=_trace)
    w_full = np.concatenate([res.results[c]["w_out"] for c in range(NCORES)], axis=0)
    b_full = np.concatenate(
        [res.results[c]["b_out"].reshape(OS) for c in range(NCORES)], axis=0
    )
    out = (w_full.astype(np.float32), b_full.astype(np.float32))
    if _trace:
        return out, res
    return out


# revision 19
# speedup vs baseline: 1.0628x; 1.0628x over previous
"""Bass/Trainium2 kernel for nn_BayesianSTDPAdaptive (8-core SPMD).

Math: the reference scan runs 400 steps, but its learning-rate state
collapses after the first step: with mu init == 1.0,
    m1 = 1*x + 0*m1_old = x,  m2 = 1*x*x + 0*m2_old = fl(x*x)
so mu = (m2 - m1*m1)/(exp(-m1)+1) = (fl(x*x) - fl(x*x))/... = exactly 0
for both the weight and bias chains (no-FMA fp32, which is what the
device-executed reference emits: separate mul/add roundings).  All later
steps then add mu*d = 0 — EXCEPT when d overflows to +/-inf, where
0*inf = NaN.  For the weights d is bounded (inputs in [0,1)), so
    w_out = w0 + (corr1 * exp(-w0) - tos1)
For the biases, step t>=1 computes db_t = (exp(-b1)*tos_t - 1)*tos_sum_t
which overflows for very negative b1; those rows become NaN.  The kernel
reproduces the NaN mask exactly by evaluating db_t in fp32 for all
t = 1..399 and testing for overflow.

Sharding: row-parallel over the 256 output neurons, 32 rows per core.
Each core reads the full output_spikes (for the per-step totals), the
first time-batch of input_psp, and its 32-row slice of weights/biases.
The host rotates each core's spike columns so its shard sits at columns
0..31 (the SPMD program is identical on every core).

Raw Bass (no Tile): the walrus build in this container rejects >1 sync
wait on a Drain instruction, which Tile's exit drain always needs, so
semaphores are managed manually: one DMA sem per input tensor (HW-DGE
queues complete out of order) and one counting sem per compute engine.
Each spike chunk is split across the two HW-DGE rings (SP + Activation),
~140 GB/s each.
"""

import sys
from contextlib import ExitStack

for _p in ("/opt/trn_rl_repo", "/root/.axon_site/_ro/trn_rl_repo"):
    if _p not in sys.path:
        sys.path.append(_p)

import numpy as np

import concourse.bass as bass
import concourse.mybir as mybir
from concourse.bass_utils import run_bass_kernel_spmd

# Problem constants (hardcoded per contract)
T, I, O = 4000, 512, 256
TB = 10
ITERS = T // TB          # 400
NCORES = 8
OS = O // NCORES         # 32 output rows per core
FLT_MAX = float(np.finfo(np.float32).max)

F32 = mybir.dt.float32
ALU = mybir.AluOpType
ACTF = mybir.ActivationFunctionType

# t-chunks over the 400 time batches; first is small so compute starts early
CHUNKS = [64, 112, 112, 112]
STARTS = [0, 64, 176, 288]

# --- engine tick numbers (each compute op incs its engine sem by 1) ---
# DVE: memsets 1-3; chunk0 (tmp,u,v,tos,tsum) 4-8; bias 9-11; mask0 12-15;
# then per k>=1 chunk 5 ops + mask 3 ops; w-chain (p, w1) last so the late
# aux/w0 DMAs never stall the chunk pipeline.
T_TOS0 = 7
T_B1 = 11
T_F = [15, 23, 31, 39]
T_P = 40
T_W1 = 41
T_BO = 44
# ACT
A_EB0, A_TOS1C, A_EBB, A_E = 1, 2, 3, 4
# PE
P_CORR, P_TOS1C, P_EBB, P_CNT = 1, 2, 3, 7


def build_nc(debug=False):
    nc = bass.Bass()

    spk = nc.declare_dram_parameter("spk", [ITERS, TB * O], F32, isOutput=False)
    # aux rows 0-9: [psp1 | spk1s]
    aux = nc.declare_dram_parameter("aux", [TB, I + OS], F32, isOutput=False)
    w0 = nc.declare_dram_parameter("w0", [OS, I], F32, isOutput=False)
    b0 = nc.declare_dram_parameter("b0", [1, OS], F32, isOutput=False)
    w_out = nc.declare_dram_parameter("w_out", [OS, I], F32, isOutput=True)
    b_out = nc.declare_dram_parameter("b_out", [1, OS], F32, isOutput=True)

    dbg = {}
    if debug:
        for name, shape in [
            ("d_st0", [128, TB * O]), ("d_tos0", [128, O]), ("d_tsum0", [128, 1]),
            ("d_e", [OS, I]), ("d_p", [OS, I]), ("d_tos1c", [OS, 1]),
            ("d_eb0", [1, OS]), ("d_y0", [1, OS]), ("d_db", [1, OS]),
            ("d_b1", [1, OS]), ("d_ebb", [128, OS]),
            ("d_yk0", [128, OS]), ("d_vk0", [128, OS]), ("d_fk0", [128, OS]),
            ("d_q", [1, OS]),
        ]:
            dbg[name] = nc.declare_dram_parameter(name, shape, F32, isOutput=True)

    es = ExitStack()
    with es:
        sb = lambda name, shape: es.enter_context(nc.sbuf_tensor(name, shape, F32))
        psumt = lambda name, shape: es.enter_context(nc.psum_tensor(name, shape, F32))

        st = [sb(f"st{k}", [128, TB * O]) for k in range(4)]
        tmp = [sb(f"tmp{k}", [128, 5 * O]) for k in range(4)]
        u = [sb(f"u{k}", [128, 2 * O]) for k in range(4)]
        v = [sb(f"v{k}", [128, O]) for k in range(4)]
        tos = [sb(f"tos{k}", [128, O]) for k in range(4)]
        tsum = [sb(f"tsum{k}", [128, 1]) for k in range(4)]

        aux_t = sb("aux_t", [TB, I + OS])
        psp1_t = aux_t[0:TB, 0:I]
        spk1s_t = aux_t[0:TB, I : I + OS]
        b0_t = sb("b0_t", [1, OS])
        w0_t = sb("w0_t", [OS, I])
        ones_row = sb("ones_row", [1, 128])
        ones_col = sb("ones_col", [128, 1])
        one_t = sb("one_t", [1, 1])

        e_t = sb("e_t", [OS, I])
        p_t = sb("p_t", [OS, I])
        w1_t = sb("w1_t", [OS, I])
        tos1c = sb("tos1c", [OS, 1])
        eb0 = sb("eb0", [1, OS])
        y0 = sb("y0", [1, OS])
        db = sb("db", [1, OS])
        b1_t = sb("b1_t", [1, OS])
        ebb = sb("ebb", [128, OS])
        yk = [sb(f"yk{k}", [128, OS]) for k in range(4)]
        vk = [sb(f"vk{k}", [128, OS]) for k in range(4)]
        fk = [sb(f"fk{k}", [128, OS]) for k in range(4)]
        q_t = sb("q_t", [1, OS])
        nanv = sb("nanv", [1, OS])
        bo_t = sb("bo_t", [1, OS])

        corr_p = psumt("corr_p", [OS, I])
        tos1c_p = psumt("tos1c_p", [OS, 1])
        b1b_p = psumt("b1b_p", [128, OS])
        cnt_p = psumt("cnt_p", [1, OS])

        sem_in = {
            name: es.enter_context(nc.semaphore(f"sem_{name}"))
            for name in ["st0", "st1", "st2", "st3", "aux", "w0", "b0"]
        }
        sem_out = es.enter_context(nc.semaphore("sem_out"))
        dve = es.enter_context(nc.semaphore("sem_dve"))
        act = es.enter_context(nc.semaphore("sem_act"))
        pe = es.enter_context(nc.semaphore("sem_pe"))
        gp = es.enter_context(nc.semaphore("sem_gp"))
        all_sems = list(sem_in.values()) + [sem_out, dve, act, pe, gp]

        ticks = {"dve": 0, "act": 0, "pe": 0, "gp": 0}
        sems = {"dve": dve, "act": act, "pe": pe, "gp": gp}

        def inc(engine_key, inst):
            ticks[engine_key] += 1
            inst.then_inc(sems[engine_key], 1)
            return ticks[engine_key]

        # each spike chunk is split between the SP and ACT HW-DGE rings;
        # both halves inc the same sem by 16, consumers wait >= 32
        def spk_dma_half(eng, k, half):
            tcnt = CHUNKS[k]
            h = tcnt // 2
            lo, hi = (0, h) if half == 0 else (h, tcnt)
            eng.dma_start(
                st[k][lo:hi, :],
                spk[STARTS[k] + lo : STARTS[k] + hi, :],
            ).then_inc(sem_in[f"st{k}"], 16)

        with nc.Block() as block:

            @block.sync
            def _(sync):
                spk_dma_half(sync, 0, 0)
                sync.dma_start(b0_t[:], b0[:]).then_inc(sem_in["b0"], 16)
                sync.dma_start(aux_t[:, :], aux[:]).then_inc(sem_in["aux"], 16)
                for k in range(1, 4):
                    spk_dma_half(sync, k, 0)
                sync.dma_start(w0_t[:], w0[:]).then_inc(sem_in["w0"], 16)
                sync.wait_ge(dve, T_W1)
                sync.dma_start(w_out[:], w1_t[:]).then_inc(sem_out, 16)
                sync.wait_ge(dve, T_BO)
                sync.dma_start(b_out[:], bo_t[:]).then_inc(sem_out, 16)
                n_out = 2
                if dbg:
                    sync.wait_ge(act, A_EBB)
                    sync.wait_ge(pe, P_CNT)
                    for name, tile in [
                        ("d_st0", st[0]), ("d_tos0", tos[0]), ("d_tsum0", tsum[0]),
                        ("d_e", e_t), ("d_p", p_t), ("d_tos1c", tos1c),
                        ("d_eb0", eb0), ("d_y0", y0), ("d_db", db),
                        ("d_b1", b1_t), ("d_ebb", ebb),
                        ("d_yk0", yk[0]), ("d_vk0", vk[0]), ("d_fk0", fk[0]),
                        ("d_q", q_t),
                    ]:
                        sync.dma_start(dbg[name][:], tile[:]).then_inc(sem_out, 16)
                        n_out += 1
                sync.wait_ge(sem_out, 16 * n_out)

            @block.vector
            def _(vector):
                # DVE is pipelined: a same-engine RAW/WAW needs a self-wait
                # on the engine sem (what Tile would auto-insert).
                def vwait():
                    vector.wait_ge(dve, ticks["dve"])

                def chunk_ops(k, tcnt):
                    vector.wait_ge(sem_in[f"st{k}"], 32)
                    inc("dve", nc.vector.tensor_tensor(
                        tmp[k][0:tcnt, :], st[k][0:tcnt, 0 : 5 * O],
                        st[k][0:tcnt, 5 * O : 10 * O], op=ALU.add))
                    vwait()
                    inc("dve", nc.vector.tensor_tensor(
                        u[k][0:tcnt, :], tmp[k][0:tcnt, 0 : 2 * O],
                        tmp[k][0:tcnt, 2 * O : 4 * O], op=ALU.add))
                    vwait()
                    inc("dve", nc.vector.tensor_tensor(
                        v[k][0:tcnt, :], u[k][0:tcnt, 0:O],
                        u[k][0:tcnt, O : 2 * O], op=ALU.add))
                    vwait()
                    inc("dve", nc.vector.tensor_tensor(
                        tos[k][0:tcnt, :], v[k][0:tcnt, :],
                        tmp[k][0:tcnt, 4 * O : 5 * O], op=ALU.add))
                    vwait()
                    inc("dve", nc.vector.tensor_reduce(
                        tsum[k][0:tcnt, :], tos[k][0:tcnt, :],
                        axis=mybir.AxisListType.X, op=ALU.add))

                def mask_ops(k, tcnt):
                    vwait()
                    inc("dve", nc.vector.tensor_tensor(
                        yk[k][0:tcnt, :], tos[k][0:tcnt, 0:OS], ebb[0:tcnt, :],
                        op=ALU.mult))
                    vwait()
                    inc("dve", nc.vector.tensor_scalar(
                        vk[k][0:tcnt, :], yk[k][0:tcnt, :], 1.0,
                        tsum[k][0:tcnt, 0:1], op0=ALU.subtract, op1=ALU.mult))
                    vwait()
                    t = inc("dve", nc.vector.tensor_single_scalar(
                        fk[k][0:tcnt, :], vk[k][0:tcnt, :], FLT_MAX, op=ALU.is_gt))
                    if k == 0:
                        # t=0 already applied via db; exclude from the mask
                        vwait()
                        t = inc("dve", nc.vector.memset(fk[0][0:1, :], 0.0))
                    assert t == T_F[k]

                t = inc("dve", nc.vector.memset(ones_row[:], 1.0)); assert t == 1
                inc("dve", nc.vector.memset(ones_col[:], 1.0))
                inc("dve", nc.vector.memset(one_t[:], 1.0))
                chunk_ops(0, CHUNKS[0])
                assert ticks["dve"] == T_TOS0 + 1
                # bias chain
                vector.wait_ge(act, A_EB0)
                vwait()
                inc("dve", nc.vector.tensor_tensor(
                    y0[:], eb0[:], tos[0][0:1, 0:OS], op=ALU.mult))
                vwait()
                inc("dve", nc.vector.tensor_scalar(
                    db[:], y0[:], 1.0, tsum[0][0:1, 0:1],
                    op0=ALU.subtract, op1=ALU.mult))
                vwait()
                t = inc("dve", nc.vector.tensor_tensor(
                    b1_t[:], b0_t[:], db[:], op=ALU.add)); assert t == T_B1
                # overflow-mask chunks, interleaved with the remaining DMAs
                vector.wait_ge(act, A_EBB)
                mask_ops(0, CHUNKS[0])
                for k in range(1, 4):
                    chunk_ops(k, CHUNKS[k])
                    mask_ops(k, CHUNKS[k])
                # w-chain last: p = corr * exp(-w0); w1 = (p - tos1) + w0
                vector.wait_ge(act, A_E)
                vector.wait_ge(pe, P_CORR)
                t = inc("dve", nc.vector.tensor_tensor(
                    p_t[:], corr_p[:], e_t[:], op=ALU.mult)); assert t == T_P
                vwait()
                t = inc("dve", nc.vector.scalar_tensor_tensor(
                    w1_t[:], p_t[:], tos1c[:, 0:1], w0_t[:],
                    op0=ALU.subtract, op1=ALU.add)); assert t == T_W1
                # nanify: count>0 -> inf -> inf-inf = NaN
                vector.wait_ge(pe, P_CNT)
                inc("dve", nc.vector.tensor_scalar(
                    q_t[:], cnt_p[:], FLT_MAX, 4.0, op0=ALU.mult, op1=ALU.mult))
                vwait()
                inc("dve", nc.vector.tensor_tensor(
                    nanv[:], q_t[:], q_t[:], op=ALU.subtract))
                vwait()
                t = inc("dve", nc.vector.tensor_tensor(
                    bo_t[:], b1_t[:], nanv[:], op=ALU.add)); assert t == T_BO

            @block.scalar
            def _(scalar):
                for k in range(4):
                    spk_dma_half(scalar, k, 1)
                scalar.wait_ge(sem_in["b0"], 16)
                t = inc("act", nc.scalar.activation(
                    eb0[:], b0_t[:], ACTF.Exp, scale=-1.0)); assert t == A_EB0
                scalar.wait_ge(pe, P_TOS1C)
                t = inc("act", nc.scalar.copy(tos1c[:], tos1c_p[:])); assert t == A_TOS1C
                scalar.wait_ge(pe, P_EBB)
                t = inc("act", nc.scalar.activation(
                    ebb[:], b1b_p[:], ACTF.Exp, scale=-1.0)); assert t == A_EBB
                scalar.wait_ge(sem_in["w0"], 16)
                t = inc("act", nc.scalar.activation(
                    e_t[:], w0_t[:], ACTF.Exp, scale=-1.0)); assert t == A_E

            @block.tensor
            def _(tensor):
                tensor.wait_ge(sem_in["aux"], 16)
                t = inc("pe", nc.tensor.matmul(
                    corr_p[:], spk1s_t, psp1_t)); assert t == P_CORR
                tensor.wait_ge(dve, T_TOS0)
                t = inc("pe", nc.tensor.matmul(
                    tos1c_p[:], tos[0][0:1, 0:OS], one_t[:])); assert t == P_TOS1C
                tensor.wait_ge(dve, T_B1)
                t = inc("pe", nc.tensor.matmul(
                    b1b_p[:], ones_row[:], b1_t[:])); assert t == P_EBB
                for k, tcnt in enumerate(CHUNKS):
                    tensor.wait_ge(dve, T_F[k])
                    t = inc("pe", nc.tensor.matmul(
                        cnt_p[:], ones_col[0:tcnt, :], fk[k][0:tcnt, :],
                        start=(k == 0), stop=(k == 3)))
                assert t == P_CNT

        # leave semaphores zeroed so a re-execution of the same loaded NEFF
        # (e.g. a second kernel() call) starts from a clean state
        nc.all_engine_barrier()
        ids = sorted(s.num for s in all_sems)
        assert ids == list(range(ids[0], ids[0] + len(ids))), ids
        nc.gpsimd.sem_clear(range(ids[0], ids[-1] + 1))
        nc.all_engine_barrier()

    return nc


_NC = None


def _get_nc():
    global _NC
    if _NC is None:
        _NC = build_nc()
    return _NC


def make_in_maps(input_psp, output_spikes, weights, biases):
    psp = np.ascontiguousarray(np.asarray(input_psp, dtype=np.float32))
    spk = np.ascontiguousarray(np.asarray(output_spikes, dtype=np.float32))
    w = np.ascontiguousarray(np.asarray(weights, dtype=np.float32))
    b = np.ascontiguousarray(np.asarray(biases, dtype=np.float32))

    in_maps = []
    for c in range(NCORES):
        r0 = c * OS
        # SPMD program is identical on every core, so rotate the spike
        # columns so this core's 32 output rows land at columns 0..31
        perm = np.concatenate(
            [np.arange(r0, r0 + OS), np.delete(np.arange(O), slice(r0, r0 + OS))]
        )
        spk_c = np.ascontiguousarray(spk[:, perm].reshape(ITERS, TB * O))
        aux = np.zeros((TB, I + OS), dtype=np.float32)
        aux[:, :I] = psp[:TB]
        aux[:, I:] = spk[:TB, r0 : r0 + OS]
        in_maps.append(
            {
                "spk": spk_c,
                "aux": aux,
                "w0": np.ascontiguousarray(w[r0 : r0 + OS]),
                "b0": np.ascontiguousarray(b[r0 : r0 + OS].reshape(1, OS)),
            }
        )
    return in_maps


def kernel(input_psp, output_spikes, weights, biases, _trace=False):
    nc = _get_nc()
    in_maps = make_in_maps(input_psp, output_spikes, weights, biases)
    res = run_bass_kernel_spmd(nc, in_maps, core_ids=list(range(NCORES)), trace=_trace)
    w_full = np.concatenate([res.results[c]["w_out"] for c in range(NCORES)], axis=0)
    b_full = np.concatenate(
        [res.results[c]["b_out"].reshape(OS) for c in range(NCORES)], axis=0
    )
    out = (w_full.astype(np.float32), b_full.astype(np.float32))
    if _trace:
        return out, res
    return out


# revision 20
# speedup vs baseline: 1.0971x; 1.0323x over previous
"""Bass/Trainium2 kernel for nn_BayesianSTDPAdaptive (8-core SPMD).

Math: the reference scan runs 400 steps, but its learning-rate state
collapses after the first step: with mu init == 1.0,
    m1 = 1*x + 0*m1_old = x,  m2 = 1*x*x + 0*m2_old = fl(x*x)
so mu = (m2 - m1*m1)/(exp(-m1)+1) = (fl(x*x) - fl(x*x))/... = exactly 0
for both the weight and bias chains (no-FMA fp32, which is what the
device-executed reference emits: separate mul/add roundings).  All later
steps then add mu*d = 0 — EXCEPT when d overflows to +/-inf, where
0*inf = NaN.  For the weights d is bounded (inputs in [0,1)), so
    w_out = w0 + (corr1 * exp(-w0) - tos1)
For the biases, step t>=1 computes db_t = (exp(-b1)*tos_t - 1)*tos_sum_t
which overflows for very negative b1; those rows become NaN.  The kernel
reproduces the NaN mask exactly by evaluating db_t in fp32 for all
t = 1..399 and testing for overflow.

Sharding: row-parallel over the 256 output neurons, 32 rows per core.
Each core reads the full output_spikes (for the per-step totals), the
first time-batch of input_psp, and its 32-row slice of weights/biases.
The host rotates each core's spike columns so its shard sits at columns
0..31 (the SPMD program is identical on every core).

Raw Bass (no Tile): the walrus build in this container rejects >1 sync
wait on a Drain instruction, which Tile's exit drain always needs, so
semaphores are managed manually: one DMA sem per input tensor (HW-DGE
queues complete out of order) and one counting sem per compute engine.
Each spike chunk is split across the two HW-DGE rings (SP + Activation),
~140 GB/s each.
"""

import sys
from contextlib import ExitStack

for _p in ("/opt/trn_rl_repo", "/root/.axon_site/_ro/trn_rl_repo"):
    if _p not in sys.path:
        sys.path.append(_p)

import numpy as np

import concourse.bass as bass
import concourse.mybir as mybir
from concourse.bass_utils import run_bass_kernel_spmd

# Problem constants (hardcoded per contract)
T, I, O = 4000, 512, 256
TB = 10
ITERS = T // TB          # 400
NCORES = 8
OS = O // NCORES         # 32 output rows per core
FLT_MAX = float(np.finfo(np.float32).max)

F32 = mybir.dt.float32
ALU = mybir.AluOpType
ACTF = mybir.ActivationFunctionType

# t-chunks over the 400 time batches; first is small so compute starts early
CHUNKS = [64, 112, 112, 112]
STARTS = [0, 64, 176, 288]

# --- engine tick numbers (each compute op incs its engine sem by 1) ---
# DVE: memset 1; chunk0 (tmp,u,v,tos,tsum) 2-6; bias 7-9; chunks 1-3 at
# 10-14/15-19/20-24; w-chain (p, w1) 25-26; q/nanv/bo 27-29.  The mask ops
# (32 els wide) run on the otherwise idle GpSimd: 3 per chunk (+1 memset).
T_TOS0 = 5
T_TSUM = [6, 14, 19, 24]
T_B1 = 9
T_P = 25
T_W1 = 26
T_BO = 29
# ACT
A_EB0, A_TOS1C, A_EBB, A_E = 1, 2, 3, 4
# PE
P_CORR, P_TOS1C, P_EBB, P_CNT = 1, 2, 3, 7
# GpSimd (masks)
G_F = [4, 7, 10, 13]


def build_nc(debug=False):
    nc = bass.Bass()

    spk = nc.declare_dram_parameter("spk", [ITERS, TB * O], F32, isOutput=False)
    # aux rows 0-9: [psp1 | spk1s]
    aux = nc.declare_dram_parameter("aux", [TB, I + OS], F32, isOutput=False)
    w0 = nc.declare_dram_parameter("w0", [OS, I], F32, isOutput=False)
    b0 = nc.declare_dram_parameter("b0", [1, OS], F32, isOutput=False)
    w_out = nc.declare_dram_parameter("w_out", [OS, I], F32, isOutput=True)
    b_out = nc.declare_dram_parameter("b_out", [1, OS], F32, isOutput=True)

    dbg = {}
    if debug:
        for name, shape in [
            ("d_st0", [128, TB * O]), ("d_tos0", [128, O]), ("d_tsum0", [128, 1]),
            ("d_e", [OS, I]), ("d_p", [OS, I]), ("d_tos1c", [OS, 1]),
            ("d_eb0", [1, OS]), ("d_y0", [1, OS]), ("d_db", [1, OS]),
            ("d_b1", [1, OS]), ("d_ebb", [128, OS]),
            ("d_yk0", [128, OS]), ("d_vk0", [128, OS]), ("d_fk0", [128, OS]),
            ("d_q", [1, OS]),
        ]:
            dbg[name] = nc.declare_dram_parameter(name, shape, F32, isOutput=True)

    es = ExitStack()
    with es:
        sb = lambda name, shape: es.enter_context(nc.sbuf_tensor(name, shape, F32))
        psumt = lambda name, shape: es.enter_context(nc.psum_tensor(name, shape, F32))

        st = [sb(f"st{k}", [128, TB * O]) for k in range(4)]
        tmp = [sb(f"tmp{k}", [128, 5 * O]) for k in range(4)]
        u = [sb(f"u{k}", [128, 2 * O]) for k in range(4)]
        v = [sb(f"v{k}", [128, O]) for k in range(4)]
        tos = [sb(f"tos{k}", [128, O]) for k in range(4)]
        tsum = [sb(f"tsum{k}", [128, 1]) for k in range(4)]

        aux_t = sb("aux_t", [TB, I + OS])
        psp1_t = aux_t[0:TB, 0:I]
        spk1s_t = aux_t[0:TB, I : I + OS]
        b0_t = sb("b0_t", [1, OS])
        w0_t = sb("w0_t", [OS, I])
        ones_big = sb("ones_big", [128, 128])
        ones_row = ones_big[0:1, :]
        ones_col = ones_big[:, 0:1]
        one_t = ones_big[0:1, 0:1]

        e_t = sb("e_t", [OS, I])
        p_t = sb("p_t", [OS, I])
        w1_t = sb("w1_t", [OS, I])
        tos1c = sb("tos1c", [OS, 1])
        eb0 = sb("eb0", [1, OS])
        y0 = sb("y0", [1, OS])
        db = sb("db", [1, OS])
        b1_t = sb("b1_t", [1, OS])
        ebb = sb("ebb", [128, OS])
        yk = [sb(f"yk{k}", [128, OS]) for k in range(4)]
        vk = [sb(f"vk{k}", [128, OS]) for k in range(4)]
        fk = [sb(f"fk{k}", [128, OS]) for k in range(4)]
        q_t = sb("q_t", [1, OS])
        nanv = sb("nanv", [1, OS])
        bo_t = sb("bo_t", [1, OS])

        corr_p = psumt("corr_p", [OS, I])
        tos1c_p = psumt("tos1c_p", [OS, 1])
        b1b_p = psumt("b1b_p", [128, OS])
        cnt_p = psumt("cnt_p", [1, OS])

        sem_in = {
            name: es.enter_context(nc.semaphore(f"sem_{name}"))
            for name in ["st0", "st1", "st2", "st3", "aux", "w0", "b0"]
        }
        sem_out = es.enter_context(nc.semaphore("sem_out"))
        dve = es.enter_context(nc.semaphore("sem_dve"))
        act = es.enter_context(nc.semaphore("sem_act"))
        pe = es.enter_context(nc.semaphore("sem_pe"))
        gp = es.enter_context(nc.semaphore("sem_gp"))
        all_sems = list(sem_in.values()) + [sem_out, dve, act, pe, gp]

        ticks = {"dve": 0, "act": 0, "pe": 0, "gp": 0}
        sems = {"dve": dve, "act": act, "pe": pe, "gp": gp}

        def inc(engine_key, inst):
            ticks[engine_key] += 1
            inst.then_inc(sems[engine_key], 1)
            return ticks[engine_key]

        # each spike chunk is split between the SP and ACT HW-DGE rings;
        # both halves inc the same sem by 16, consumers wait >= 32
        def spk_dma_half(eng, k, half):
            tcnt = CHUNKS[k]
            h = tcnt // 2
            lo, hi = (0, h) if half == 0 else (h, tcnt)
            eng.dma_start(
                st[k][lo:hi, :],
                spk[STARTS[k] + lo : STARTS[k] + hi, :],
            ).then_inc(sem_in[f"st{k}"], 16)

        with nc.Block() as block:

            @block.sync
            def _(sync):
                spk_dma_half(sync, 0, 0)
                sync.dma_start(b0_t[:], b0[:]).then_inc(sem_in["b0"], 16)
                sync.dma_start(aux_t[:, :], aux[:]).then_inc(sem_in["aux"], 16)
                for k in range(1, 4):
                    spk_dma_half(sync, k, 0)
                sync.dma_start(w0_t[:], w0[:]).then_inc(sem_in["w0"], 16)
                sync.wait_ge(dve, T_W1)
                sync.dma_start(w_out[:], w1_t[:]).then_inc(sem_out, 16)
                sync.wait_ge(dve, T_BO)
                sync.dma_start(b_out[:], bo_t[:]).then_inc(sem_out, 16)
                n_out = 2
                if dbg:
                    sync.wait_ge(act, A_EBB)
                    sync.wait_ge(pe, P_CNT)
                    for name, tile in [
                        ("d_st0", st[0]), ("d_tos0", tos[0]), ("d_tsum0", tsum[0]),
                        ("d_e", e_t), ("d_p", p_t), ("d_tos1c", tos1c),
                        ("d_eb0", eb0), ("d_y0", y0), ("d_db", db),
                        ("d_b1", b1_t), ("d_ebb", ebb),
                        ("d_yk0", yk[0]), ("d_vk0", vk[0]), ("d_fk0", fk[0]),
                        ("d_q", q_t),
                    ]:
                        sync.dma_start(dbg[name][:], tile[:]).then_inc(sem_out, 16)
                        n_out += 1
                sync.wait_ge(sem_out, 16 * n_out)

            @block.vector
            def _(vector):
                # DVE is pipelined: a same-engine RAW/WAW needs a self-wait
                # on the engine sem (what Tile would auto-insert).
                def vwait():
                    vector.wait_ge(dve, ticks["dve"])

                def chunk_ops(k, tcnt):
                    vector.wait_ge(sem_in[f"st{k}"], 32)
                    inc("dve", nc.vector.tensor_tensor(
                        tmp[k][0:tcnt, :], st[k][0:tcnt, 0 : 5 * O],
                        st[k][0:tcnt, 5 * O : 10 * O], op=ALU.add))
                    vwait()
                    inc("dve", nc.vector.tensor_tensor(
                        u[k][0:tcnt, :], tmp[k][0:tcnt, 0 : 2 * O],
                        tmp[k][0:tcnt, 2 * O : 4 * O], op=ALU.add))
                    vwait()
                    inc("dve", nc.vector.tensor_tensor(
                        v[k][0:tcnt, :], u[k][0:tcnt, 0:O],
                        u[k][0:tcnt, O : 2 * O], op=ALU.add))
                    vwait()
                    inc("dve", nc.vector.tensor_tensor(
                        tos[k][0:tcnt, :], v[k][0:tcnt, :],
                        tmp[k][0:tcnt, 4 * O : 5 * O], op=ALU.add))
                    vwait()
                    t = inc("dve", nc.vector.tensor_reduce(
                        tsum[k][0:tcnt, :], tos[k][0:tcnt, :],
                        axis=mybir.AxisListType.X, op=ALU.add))
                    assert t == T_TSUM[k]

                t = inc("dve", nc.vector.memset(ones_big[:, :], 1.0)); assert t == 1
                chunk_ops(0, CHUNKS[0])
                # bias chain
                vector.wait_ge(act, A_EB0)
                vwait()
                inc("dve", nc.vector.tensor_tensor(
                    y0[:], eb0[:], tos[0][0:1, 0:OS], op=ALU.mult))
                vwait()
                inc("dve", nc.vector.tensor_scalar(
                    db[:], y0[:], 1.0, tsum[0][0:1, 0:1],
                    op0=ALU.subtract, op1=ALU.mult))
                vwait()
                t = inc("dve", nc.vector.tensor_tensor(
                    b1_t[:], b0_t[:], db[:], op=ALU.add)); assert t == T_B1
                for k in range(1, 4):
                    chunk_ops(k, CHUNKS[k])
                # w-chain: p = corr * exp(-w0); w1 = (p - tos1) + w0
                vector.wait_ge(act, A_E)
                vector.wait_ge(pe, P_CORR)
                t = inc("dve", nc.vector.tensor_tensor(
                    p_t[:], corr_p[:], e_t[:], op=ALU.mult)); assert t == T_P
                vector.wait_ge(act, A_TOS1C)
                vwait()
                t = inc("dve", nc.vector.scalar_tensor_tensor(
                    w1_t[:], p_t[:], tos1c[:, 0:1], w0_t[:],
                    op0=ALU.subtract, op1=ALU.add)); assert t == T_W1
                # nanify: count>0 -> inf -> inf-inf = NaN
                vector.wait_ge(pe, P_CNT)
                inc("dve", nc.vector.tensor_scalar(
                    q_t[:], cnt_p[:], FLT_MAX, 4.0, op0=ALU.mult, op1=ALU.mult))
                vwait()
                inc("dve", nc.vector.tensor_tensor(
                    nanv[:], q_t[:], q_t[:], op=ALU.subtract))
                vwait()
                t = inc("dve", nc.vector.tensor_tensor(
                    bo_t[:], b1_t[:], nanv[:], op=ALU.add)); assert t == T_BO

            @block.gpsimd
            def _(gpsimd):
                # overflow-mask chains on the otherwise idle GpSimd
                def gwait():
                    gpsimd.wait_ge(gp, ticks["gp"])

                gpsimd.wait_ge(act, A_EBB)
                for k, tcnt in enumerate(CHUNKS):
                    gpsimd.wait_ge(dve, T_TSUM[k])
                    if k > 0:
                        gwait()
                    inc("gp", nc.gpsimd.tensor_mul(
                        yk[k][0:tcnt, :], tos[k][0:tcnt, 0:OS], ebb[0:tcnt, :]))
                    gwait()
                    inc("gp", nc.gpsimd.tensor_scalar(
                        vk[k][0:tcnt, :], yk[k][0:tcnt, :], 1.0,
                        tsum[k][0:tcnt, 0:1], op0=ALU.subtract, op1=ALU.mult))
                    gwait()
                    t = inc("gp", nc.gpsimd.tensor_scalar(
                        fk[k][0:tcnt, :], vk[k][0:tcnt, :], FLT_MAX, None,
                        op0=ALU.is_gt))
                    if k == 0:
                        # t=0 already applied via db; exclude from the mask
                        gwait()
                        t = inc("gp", nc.gpsimd.memset(fk[0][0:1, :], 0.0))
                    assert t == G_F[k]

            @block.scalar
            def _(scalar):
                for k in range(4):
                    spk_dma_half(scalar, k, 1)
                scalar.wait_ge(sem_in["b0"], 16)
                t = inc("act", nc.scalar.activation(
                    eb0[:], b0_t[:], ACTF.Exp, scale=-1.0)); assert t == A_EB0
                scalar.wait_ge(pe, P_TOS1C)
                t = inc("act", nc.scalar.copy(tos1c[:], tos1c_p[:])); assert t == A_TOS1C
                scalar.wait_ge(pe, P_EBB)
                t = inc("act", nc.scalar.activation(
                    ebb[:], b1b_p[:], ACTF.Exp, scale=-1.0)); assert t == A_EBB
                scalar.wait_ge(sem_in["w0"], 16)
                t = inc("act", nc.scalar.activation(
                    e_t[:], w0_t[:], ACTF.Exp, scale=-1.0)); assert t == A_E

            @block.tensor
            def _(tensor):
                tensor.wait_ge(sem_in["aux"], 16)
                t = inc("pe", nc.tensor.matmul(
                    corr_p[:], spk1s_t, psp1_t)); assert t == P_CORR
                tensor.wait_ge(dve, T_TOS0)
                t = inc("pe", nc.tensor.matmul(
                    tos1c_p[:], tos[0][0:1, 0:OS], one_t[:])); assert t == P_TOS1C
                tensor.wait_ge(dve, T_B1)
                t = inc("pe", nc.tensor.matmul(
                    b1b_p[:], ones_row[:], b1_t[:])); assert t == P_EBB
                for k, tcnt in enumerate(CHUNKS):
                    tensor.wait_ge(gp, G_F[k])
                    t = inc("pe", nc.tensor.matmul(
                        cnt_p[:], ones_col[0:tcnt, :], fk[k][0:tcnt, :],
                        start=(k == 0), stop=(k == 3)))
                assert t == P_CNT

        # leave semaphores zeroed so a re-execution of the same loaded NEFF
        # (e.g. a second kernel() call) starts from a clean state
        nc.all_engine_barrier()
        ids = sorted(s.num for s in all_sems)
        assert ids == list(range(ids[0], ids[0] + len(ids))), ids
        nc.gpsimd.sem_clear(range(ids[0], ids[-1] + 1))
        nc.all_engine_barrier()

    return nc


_NC = None


def _get_nc():
    global _NC
    if _NC is None:
        _NC = build_nc()
    return _NC


def make_in_maps(input_psp, output_spikes, weights, biases):
    psp = np.ascontiguousarray(np.asarray(input_psp, dtype=np.float32))
    spk = np.ascontiguousarray(np.asarray(output_spikes, dtype=np.float32))
    w = np.ascontiguousarray(np.asarray(weights, dtype=np.float32))
    b = np.ascontiguousarray(np.asarray(biases, dtype=np.float32))

    in_maps = []
    for c in range(NCORES):
        r0 = c * OS
        # SPMD program is identical on every core, so rotate the spike
        # columns so this core's 32 output rows land at columns 0..31
        perm = np.concatenate(
            [np.arange(r0, r0 + OS), np.delete(np.arange(O), slice(r0, r0 + OS))]
        )
        spk_c = np.ascontiguousarray(spk[:, perm].reshape(ITERS, TB * O))
        aux = np.zeros((TB, I + OS), dtype=np.float32)
        aux[:, :I] = psp[:TB]
        aux[:, I:] = spk[:TB, r0 : r0 + OS]
        in_maps.append(
            {
                "spk": spk_c,
                "aux": aux,
                "w0": np.ascontiguousarray(w[r0 : r0 + OS]),
                "b0": np.ascontiguousarray(b[r0 : r0 + OS].reshape(1, OS)),
            }
        )
    return in_maps


def kernel(input_psp, output_spikes, weights, biases, _trace=False):
    nc = _get_nc()
    in_maps = make_in_maps(input_psp, output_spikes, weights, biases)
    res = run_bass_kernel_spmd(nc, in_maps, core_ids=list(range(NCORES)), trace=_trace)
    w_full = np.concatenate([res.results[c]["w_out"] for c in range(NCORES)], axis=0)
    b_full = np.concatenate(
        [res.results[c]["b_out"].reshape(OS) for c in range(NCORES)], axis=0
    )
    out = (w_full.astype(np.float32), b_full.astype(np.float32))
    if _trace:
        return out, res
    return out


# revision 21
# speedup vs baseline: 1.1385x; 1.0377x over previous
"""Bass/Trainium2 kernel for nn_BayesianSTDPAdaptive (8-core SPMD).

Math: the reference scan runs 400 steps, but its learning-rate state
collapses after the first step: with mu init == 1.0,
    m1 = 1*x + 0*m1_old = x,  m2 = 1*x*x + 0*m2_old = fl(x*x)
so mu = (m2 - m1*m1)/(exp(-m1)+1) = (fl(x*x) - fl(x*x))/... = exactly 0
for both the weight and bias chains (no-FMA fp32, which is what the
device-executed reference emits: separate mul/add roundings).  All later
steps then add mu*d = 0 — EXCEPT when d overflows to +/-inf, where
0*inf = NaN.  For the weights d is bounded (inputs in [0,1)), so
    w_out = w0 + (corr1 * exp(-w0) - tos1)
For the biases, step t>=1 computes db_t = (exp(-b1)*tos_t - 1)*tos_sum_t
which overflows for very negative b1; those rows become NaN.  The kernel
reproduces the NaN mask exactly by evaluating db_t in fp32 for all
t = 1..399 and testing for overflow.

Sharding: row-parallel over the 256 output neurons, 32 rows per core.
Each core reads the full output_spikes (for the per-step totals), the
first time-batch of input_psp, and its 32-row slice of weights/biases.
The host rotates each core's spike columns so its shard sits at columns
0..31 (the SPMD program is identical on every core).

Raw Bass (no Tile): the walrus build in this container rejects >1 sync
wait on a Drain instruction, which Tile's exit drain always needs, so
semaphores are managed manually: one DMA sem per input tensor (HW-DGE
queues complete out of order) and one counting sem per compute engine.
Each spike chunk is split across the two HW-DGE rings (SP + Activation),
~140 GB/s each.
"""

import sys
from contextlib import ExitStack

for _p in ("/opt/trn_rl_repo", "/root/.axon_site/_ro/trn_rl_repo"):
    if _p not in sys.path:
        sys.path.append(_p)

import numpy as np

import concourse.bass as bass
import concourse.mybir as mybir
from concourse.bass_utils import run_bass_kernel_spmd

# Problem constants (hardcoded per contract)
T, I, O = 4000, 512, 256
TB = 10
ITERS = T // TB          # 400
NCORES = 8
OS = O // NCORES         # 32 output rows per core
FLT_MAX = float(np.finfo(np.float32).max)

F32 = mybir.dt.float32
ALU = mybir.AluOpType
ACTF = mybir.ActivationFunctionType

# t-chunks over the 400 time batches; first is small so compute starts early
CHUNKS = [48, 112, 120, 120]
STARTS = [0, 48, 160, 280]

# --- engine tick numbers (each compute op incs its engine sem by 1) ---
# DVE: memset 1; chunk0 (tmp,u,v,tos,tsum) 2-6; bias 7-9; chunks 1-3 at
# 10-14/15-19/20-24; w-chain (p, w1) 25-26; q/nanv/bo 27-29.  The mask ops
# (32 els wide) run on the otherwise idle GpSimd: 3 per chunk (+1 memset).
T_TOS0 = 5
T_TSUM = [6, 14, 19, 24]
T_B1 = 9
T_P = 25
T_W1 = 26
T_BO = 29
# ACT
A_EB0, A_TOS1C, A_EBB, A_E = 1, 2, 3, 4
# PE
P_CORR, P_TOS1C, P_EBB, P_CNT = 1, 2, 3, 7
# GpSimd (masks)
G_F = [4, 7, 10, 13]


def build_nc(debug=False):
    nc = bass.Bass()

    spk = nc.declare_dram_parameter("spk", [ITERS, TB * O], F32, isOutput=False)
    # aux rows 0-9: [psp1 | spk1s]
    aux = nc.declare_dram_parameter("aux", [TB, I + OS], F32, isOutput=False)
    w0 = nc.declare_dram_parameter("w0", [OS, I], F32, isOutput=False)
    b0 = nc.declare_dram_parameter("b0", [1, OS], F32, isOutput=False)
    w_out = nc.declare_dram_parameter("w_out", [OS, I], F32, isOutput=True)
    b_out = nc.declare_dram_parameter("b_out", [1, OS], F32, isOutput=True)

    dbg = {}
    if debug:
        for name, shape in [
            ("d_st0", [128, TB * O]), ("d_tos0", [128, O]), ("d_tsum0", [128, 1]),
            ("d_e", [OS, I]), ("d_p", [OS, I]), ("d_tos1c", [OS, 1]),
            ("d_eb0", [1, OS]), ("d_y0", [1, OS]), ("d_db", [1, OS]),
            ("d_b1", [1, OS]), ("d_ebb", [128, OS]),
            ("d_yk0", [128, OS]), ("d_vk0", [128, OS]), ("d_fk0", [128, OS]),
            ("d_q", [1, OS]),
        ]:
            dbg[name] = nc.declare_dram_parameter(name, shape, F32, isOutput=True)

    es = ExitStack()
    with es:
        sb = lambda name, shape: es.enter_context(nc.sbuf_tensor(name, shape, F32))
        psumt = lambda name, shape: es.enter_context(nc.psum_tensor(name, shape, F32))

        st = [sb(f"st{k}", [128, TB * O]) for k in range(4)]
        tmp = [sb(f"tmp{k}", [128, 5 * O]) for k in range(4)]
        u = [sb(f"u{k}", [128, 2 * O]) for k in range(4)]
        v = [sb(f"v{k}", [128, O]) for k in range(4)]
        tos = [sb(f"tos{k}", [128, O]) for k in range(4)]
        tsum = [sb(f"tsum{k}", [128, 1]) for k in range(4)]

        aux_t = sb("aux_t", [TB, I + OS])
        psp1_t = aux_t[0:TB, 0:I]
        spk1s_t = aux_t[0:TB, I : I + OS]
        b0_t = sb("b0_t", [1, OS])
        w0_t = sb("w0_t", [OS, I])
        ones_big = sb("ones_big", [128, 128])
        ones_row = ones_big[0:1, :]
        ones_col = ones_big[:, 0:1]
        one_t = ones_big[0:1, 0:1]

        e_t = sb("e_t", [OS, I])
        p_t = sb("p_t", [OS, I])
        w1_t = sb("w1_t", [OS, I])
        tos1c = sb("tos1c", [OS, 1])
        eb0 = sb("eb0", [1, OS])
        y0 = sb("y0", [1, OS])
        db = sb("db", [1, OS])
        b1_t = sb("b1_t", [1, OS])
        ebb = sb("ebb", [128, OS])
        yk = [sb(f"yk{k}", [128, OS]) for k in range(4)]
        vk = [sb(f"vk{k}", [128, OS]) for k in range(4)]
        fk = [sb(f"fk{k}", [128, OS]) for k in range(4)]
        q_t = sb("q_t", [1, OS])
        nanv = sb("nanv", [1, OS])
        bo_t = sb("bo_t", [1, OS])

        corr_p = psumt("corr_p", [OS, I])
        tos1c_p = psumt("tos1c_p", [OS, 1])
        b1b_p = psumt("b1b_p", [128, OS])
        cnt_p = psumt("cnt_p", [1, OS])

        sem_in = {
            name: es.enter_context(nc.semaphore(f"sem_{name}"))
            for name in ["st0", "st1", "st2", "st3", "aux", "w0", "b0"]
        }
        sem_out = es.enter_context(nc.semaphore("sem_out"))
        dve = es.enter_context(nc.semaphore("sem_dve"))
        act = es.enter_context(nc.semaphore("sem_act"))
        pe = es.enter_context(nc.semaphore("sem_pe"))
        gp = es.enter_context(nc.semaphore("sem_gp"))
        all_sems = list(sem_in.values()) + [sem_out, dve, act, pe, gp]

        ticks = {"dve": 0, "act": 0, "pe": 0, "gp": 0}
        sems = {"dve": dve, "act": act, "pe": pe, "gp": gp}

        def inc(engine_key, inst):
            ticks[engine_key] += 1
            inst.then_inc(sems[engine_key], 1)
            return ticks[engine_key]

        # each spike chunk is split between the SP and ACT HW-DGE rings;
        # both halves inc the same sem by 16, consumers wait >= 32
        def spk_dma_half(eng, k, half):
            tcnt = CHUNKS[k]
            h = tcnt // 2
            lo, hi = (0, h) if half == 0 else (h, tcnt)
            eng.dma_start(
                st[k][lo:hi, :],
                spk[STARTS[k] + lo : STARTS[k] + hi, :],
            ).then_inc(sem_in[f"st{k}"], 16)

        with nc.Block() as block:

            @block.sync
            def _(sync):
                spk_dma_half(sync, 0, 0)
                sync.dma_start(b0_t[:], b0[:]).then_inc(sem_in["b0"], 16)
                sync.dma_start(aux_t[:, :], aux[:]).then_inc(sem_in["aux"], 16)
                for k in range(1, 4):
                    spk_dma_half(sync, k, 0)
                sync.dma_start(w0_t[:], w0[:]).then_inc(sem_in["w0"], 16)
                sync.wait_ge(dve, T_W1)
                sync.dma_start(w_out[:], w1_t[:]).then_inc(sem_out, 16)
                sync.wait_ge(dve, T_BO)
                sync.dma_start(b_out[:], bo_t[:]).then_inc(sem_out, 16)
                n_out = 2
                if dbg:
                    sync.wait_ge(act, A_EBB)
                    sync.wait_ge(pe, P_CNT)
                    for name, tile in [
                        ("d_st0", st[0]), ("d_tos0", tos[0]), ("d_tsum0", tsum[0]),
                        ("d_e", e_t), ("d_p", p_t), ("d_tos1c", tos1c),
                        ("d_eb0", eb0), ("d_y0", y0), ("d_db", db),
                        ("d_b1", b1_t), ("d_ebb", ebb),
                        ("d_yk0", yk[0]), ("d_vk0", vk[0]), ("d_fk0", fk[0]),
                        ("d_q", q_t),
                    ]:
                        sync.dma_start(dbg[name][:], tile[:]).then_inc(sem_out, 16)
                        n_out += 1
                sync.wait_ge(sem_out, 16 * n_out)

            @block.vector
            def _(vector):
                # DVE is pipelined: a same-engine RAW/WAW needs a self-wait
                # on the engine sem (what Tile would auto-insert).
                def vwait():
                    vector.wait_ge(dve, ticks["dve"])

                def chunk_ops(k, tcnt):
                    vector.wait_ge(sem_in[f"st{k}"], 32)
                    inc("dve", nc.vector.tensor_tensor(
                        tmp[k][0:tcnt, :], st[k][0:tcnt, 0 : 5 * O],
                        st[k][0:tcnt, 5 * O : 10 * O], op=ALU.add))
                    vwait()
                    inc("dve", nc.vector.tensor_tensor(
                        u[k][0:tcnt, :], tmp[k][0:tcnt, 0 : 2 * O],
                        tmp[k][0:tcnt, 2 * O : 4 * O], op=ALU.add))
                    vwait()
                    inc("dve", nc.vector.tensor_tensor(
                        v[k][0:tcnt, :], u[k][0:tcnt, 0:O],
                        u[k][0:tcnt, O : 2 * O], op=ALU.add))
                    vwait()
                    inc("dve", nc.vector.tensor_tensor(
                        tos[k][0:tcnt, :], v[k][0:tcnt, :],
                        tmp[k][0:tcnt, 4 * O : 5 * O], op=ALU.add))
                    vwait()
                    t = inc("dve", nc.vector.tensor_reduce(
                        tsum[k][0:tcnt, :], tos[k][0:tcnt, :],
                        axis=mybir.AxisListType.X, op=ALU.add))
                    assert t == T_TSUM[k]

                t = inc("dve", nc.vector.memset(ones_big[:, :], 1.0)); assert t == 1
                chunk_ops(0, CHUNKS[0])
                # bias chain
                vector.wait_ge(act, A_EB0)
                vwait()
                inc("dve", nc.vector.tensor_tensor(
                    y0[:], eb0[:], tos[0][0:1, 0:OS], op=ALU.mult))
                vwait()
                inc("dve", nc.vector.tensor_scalar(
                    db[:], y0[:], 1.0, tsum[0][0:1, 0:1],
                    op0=ALU.subtract, op1=ALU.mult))
                vwait()
                t = inc("dve", nc.vector.tensor_tensor(
                    b1_t[:], b0_t[:], db[:], op=ALU.add)); assert t == T_B1
                for k in range(1, 4):
                    chunk_ops(k, CHUNKS[k])
                # w-chain: p = corr * exp(-w0); w1 = (p - tos1) + w0
                vector.wait_ge(act, A_E)
                vector.wait_ge(pe, P_CORR)
                t = inc("dve", nc.vector.tensor_tensor(
                    p_t[:], corr_p[:], e_t[:], op=ALU.mult)); assert t == T_P
                vector.wait_ge(act, A_TOS1C)
                vwait()
                t = inc("dve", nc.vector.scalar_tensor_tensor(
                    w1_t[:], p_t[:], tos1c[:, 0:1], w0_t[:],
                    op0=ALU.subtract, op1=ALU.add)); assert t == T_W1
                # nanify: count>0 -> inf -> inf-inf = NaN
                vector.wait_ge(pe, P_CNT)
                inc("dve", nc.vector.tensor_scalar(
                    q_t[:], cnt_p[:], FLT_MAX, 4.0, op0=ALU.mult, op1=ALU.mult))
                vwait()
                inc("dve", nc.vector.tensor_tensor(
                    nanv[:], q_t[:], q_t[:], op=ALU.subtract))
                vwait()
                t = inc("dve", nc.vector.tensor_tensor(
                    bo_t[:], b1_t[:], nanv[:], op=ALU.add)); assert t == T_BO

            @block.gpsimd
            def _(gpsimd):
                # overflow-mask chains on the otherwise idle GpSimd
                def gwait():
                    gpsimd.wait_ge(gp, ticks["gp"])

                gpsimd.wait_ge(act, A_EBB)
                for k, tcnt in enumerate(CHUNKS):
                    gpsimd.wait_ge(dve, T_TSUM[k])
                    if k > 0:
                        gwait()
                    inc("gp", nc.gpsimd.tensor_mul(
                        yk[k][0:tcnt, :], tos[k][0:tcnt, 0:OS], ebb[0:tcnt, :]))
                    gwait()
                    inc("gp", nc.gpsimd.tensor_scalar(
                        vk[k][0:tcnt, :], yk[k][0:tcnt, :], 1.0,
                        tsum[k][0:tcnt, 0:1], op0=ALU.subtract, op1=ALU.mult))
                    gwait()
                    t = inc("gp", nc.gpsimd.tensor_scalar(
                        fk[k][0:tcnt, :], vk[k][0:tcnt, :], FLT_MAX, None,
                        op0=ALU.is_gt))
                    if k == 0:
                        # t=0 already applied via db; exclude from the mask
                        gwait()
                        t = inc("gp", nc.gpsimd.memset(fk[0][0:1, :], 0.0))
                    assert t == G_F[k]

            @block.scalar
            def _(scalar):
                for k in range(4):
                    spk_dma_half(scalar, k, 1)
                scalar.wait_ge(sem_in["b0"], 16)
                t = inc("act", nc.scalar.activation(
                    eb0[:], b0_t[:], ACTF.Exp, scale=-1.0)); assert t == A_EB0
                scalar.wait_ge(pe, P_TOS1C)
                t = inc("act", nc.scalar.copy(tos1c[:], tos1c_p[:])); assert t == A_TOS1C
                scalar.wait_ge(pe, P_EBB)
                t = inc("act", nc.scalar.activation(
                    ebb[:], b1b_p[:], ACTF.Exp, scale=-1.0)); assert t == A_EBB
                scalar.wait_ge(sem_in["w0"], 16)
                t = inc("act", nc.scalar.activation(
                    e_t[:], w0_t[:], ACTF.Exp, scale=-1.0)); assert t == A_E

            @block.tensor
            def _(tensor):
                tensor.wait_ge(sem_in["aux"], 16)
                t = inc("pe", nc.tensor.matmul(
                    corr_p[:], spk1s_t, psp1_t)); assert t == P_CORR
                tensor.wait_ge(dve, T_TOS0)
                t = inc("pe", nc.tensor.matmul(
                    tos1c_p[:], tos[0][0:1, 0:OS], one_t[:])); assert t == P_TOS1C
                tensor.wait_ge(dve, T_B1)
                t = inc("pe", nc.tensor.matmul(
                    b1b_p[:], ones_row[:], b1_t[:])); assert t == P_EBB
                for k, tcnt in enumerate(CHUNKS):
                    tensor.wait_ge(gp, G_F[k])
                    t = inc("pe", nc.tensor.matmul(
                        cnt_p[:], ones_col[0:tcnt, :], fk[k][0:tcnt, :],
                        start=(k == 0), stop=(k == 3)))
                assert t == P_CNT

        # leave semaphores zeroed so a re-execution of the same loaded NEFF
        # starts clean; the Block exit above already barriers all engines,
        # and gpsimd halts only after this clear
        ids = sorted(s.num for s in all_sems)
        assert ids == list(range(ids[0], ids[0] + len(ids))), ids
        nc.gpsimd.sem_clear(range(ids[0], ids[-1] + 1))

    return nc


_NC = None


def _get_nc():
    global _NC
    if _NC is None:
        _NC = build_nc()
    return _NC


def make_in_maps(input_psp, output_spikes, weights, biases):
    psp = np.ascontiguousarray(np.asarray(input_psp, dtype=np.float32))
    spk = np.ascontiguousarray(np.asarray(output_spikes, dtype=np.float32))
    w = np.ascontiguousarray(np.asarray(weights, dtype=np.float32))
    b = np.ascontiguousarray(np.asarray(biases, dtype=np.float32))

    in_maps = []
    for c in range(NCORES):
        r0 = c * OS
        # SPMD program is identical on every core, so rotate the spike
        # columns so this core's 32 output rows land at columns 0..31
        perm = np.concatenate(
            [np.arange(r0, r0 + OS), np.delete(np.arange(O), slice(r0, r0 + OS))]
        )
        spk_c = np.ascontiguousarray(spk[:, perm].reshape(ITERS, TB * O))
        aux = np.zeros((TB, I + OS), dtype=np.float32)
        aux[:, :I] = psp[:TB]
        aux[:, I:] = spk[:TB, r0 : r0 + OS]
        in_maps.append(
            {
                "spk": spk_c,
                "aux": aux,
                "w0": np.ascontiguousarray(w[r0 : r0 + OS]),
                "b0": np.ascontiguousarray(b[r0 : r0 + OS].reshape(1, OS)),
            }
        )
    return in_maps


def kernel(input_psp, output_spikes, weights, biases, _trace=False):
    nc = _get_nc()
    in_maps = make_in_maps(input_psp, output_spikes, weights, biases)
    res = run_bass_kernel_spmd(nc, in_maps, core_ids=list(range(NCORES)), trace=_trace)
    w_full = np.concatenate([res.results[c]["w_out"] for c in range(NCORES)], axis=0)
    b_full = np.concatenate(
        [res.results[c]["b_out"].reshape(OS) for c in range(NCORES)], axis=0
    )
    out = (w_full.astype(np.float32), b_full.astype(np.float32))
    if _trace:
        return out, res
    return out
